# revision 4
# baseline (speedup 1.0000x reference)
"""DGCNN prediction head on 8 Trainium2 NeuronCores — v2.

Data-parallel over batch B=8, one sample per core. Per-core redesign vs v1:
  - ranking R = 2<xi,xj> - ||xj||^2 + 768 computed in f32r matmuls (extra
    contract rows fold the -||xj||^2 and +768 terms); positive floats compare
    like u32 bit patterns.
  - top-20: gpsimd packs a 9-bit segment-local iota into the low mantissa
    bits straight out of PSUM; DVE does 8 segmented max8 (exact top-8 per
    512-col segment), then 3 cheap rounds on the 64 candidates; indices
    decode from packed bits + candidate position. One multi-index indirect
    DMA gathers all 20 neighbor rows per point.
  - EdgeConv2 + both MLP lrelus use W*lrelu(z) = W*(0.6 z) + W*(0.4|z|) with
    scaled Identity/Abs PSUM drains, so no elementwise lrelu pass is needed.
  - neighbor max via DVE tensor_reduce over a strided k-view.
  - 32 point-blocks are software-pipelined across PE/DVE/GPSIMD/Act engines.
"""

import numpy as np

C = 64
K = 20
NEG = 0.2
EPS = 1e-5
NCORES = 8
N_FULL = 4096
OFF = 768.0
NEG_FILL = 0.0

_cache = {}


def build_nc(n):
    from contextlib import ExitStack

    import concourse.bass as bass
    import concourse.bacc as bacc
    import concourse.mybir as mybir
    import concourse.tile as tile
    from concourse.masks import make_identity

    f32 = mybir.dt.float32
    f32r = mybir.dt.float32r
    u32 = mybir.dt.uint32
    AF = mybir.ActivationFunctionType
    OP = mybir.AluOpType
    AX = mybir.AxisListType

    nblk = n // 128
    nchk = n // 512

    nc = bacc.Bacc("TRN2", target_bir_lowering=False, debug=False,
                   num_devices=NCORES)

    x_d = nc.dram_tensor("x", [C, n], f32, kind="ExternalInput")
    wnT_d = nc.dram_tensor("wnT", [C, C], f32, kind="ExternalInput")
    wcnT_d = nc.dram_tensor("wcnT", [C, C], f32, kind="ExternalInput")
    t1_d = nc.dram_tensor("t1", [C, 1], f32, kind="ExternalInput")
    w2T_d = nc.dram_tensor("w2T", [C, C], f32, kind="ExternalInput")
    t2_d = nc.dram_tensor("t2", [C, 1], f32, kind="ExternalInput")
    w1aT_d = nc.dram_tensor("w1aT", [C, 128], f32, kind="ExternalInput")
    w1bT_d = nc.dram_tensor("w1bT", [C, 128], f32, kind="ExternalInput")
    tm1a06_d = nc.dram_tensor("tm1a06", [128, 1], f32, kind="ExternalInput")
    tm1a04_d = nc.dram_tensor("tm1a04", [128, 1], f32, kind="ExternalInput")
    tm1b06_d = nc.dram_tensor("tm1b06", [128, 1], f32, kind="ExternalInput")
    tm1b04_d = nc.dram_tensor("tm1b04", [128, 1], f32, kind="ExternalInput")
    w2maT_d = nc.dram_tensor("w2maT", [128, 128], f32, kind="ExternalInput")
    w2mbT_d = nc.dram_tensor("w2mbT", [128, 128], f32, kind="ExternalInput")
    tm206_d = nc.dram_tensor("tm206", [128, 1], f32, kind="ExternalInput")
    tm204_d = nc.dram_tensor("tm204", [128, 1], f32, kind="ExternalInput")
    w3T_d = nc.dram_tensor("w3T", [128, 1], f32, kind="ExternalInput")
    b3_d = nc.dram_tensor("b3", [1, 1], f32, kind="ExternalInput")
    augb1_d = nc.dram_tensor("augb1", [128, 1], f32, kind="ExternalInput")
    augb2_d = nc.dram_tensor("augb2", [128, 1], f32, kind="ExternalInput")
    out_d = nc.dram_tensor("out", [1, n], f32, kind="ExternalOutput")

    with tile.TileContext(nc) as tc, ExitStack() as top:
        cpool = top.enter_context(tc.tile_pool(name="consts", bufs=1))
        dpool = top.enter_context(tc.tile_pool(name="dram", bufs=1, space="DRAM"))
        xpool = top.enter_context(tc.tile_pool(name="xaug", bufs=1))
        hpool = top.enter_context(tc.tile_pool(name="hout", bufs=1))

        ident = cpool.tile([128, 128], f32, tag="ident")
        make_identity(nc, ident[:])
        i2 = cpool.tile([C, 128], f32, tag="i2")  # [I64 | I64]
        make_identity(nc, i2[:, :C])
        make_identity(nc, i2[:, C:])
        ones64 = cpool.tile([C, 1], f32, tag="ones64")
        nc.vector.memset(ones64[:], 1.0)

        def load_const(dram, shape, tag):
            t = cpool.tile(shape, f32, tag=tag)
            nc.sync.dma_start(t[:], dram[:])
            return t

        wnT = load_const(wnT_d, [C, C], "wnT")
        wcnT = load_const(wcnT_d, [C, C], "wcnT")
        t1 = load_const(t1_d, [C, 1], "t1")
        w2T = cpool.tile([128, C], f32, tag="w2T")   # duplicated in both halves
        nc.sync.dma_start(w2T[:C, :], w2T_d[:])
        nc.sync.dma_start(w2T[C:, :], w2T_d[:])
        t2 = load_const(t2_d, [C, 1], "t2")
        w1aT = load_const(w1aT_d, [C, 128], "w1aT")
        w1bT = load_const(w1bT_d, [C, 128], "w1bT")
        tm1a06 = load_const(tm1a06_d, [128, 1], "tm1a06")
        tm1a04 = load_const(tm1a04_d, [128, 1], "tm1a04")
        tm1b06 = load_const(tm1b06_d, [128, 1], "tm1b06")
        tm1b04 = load_const(tm1b04_d, [128, 1], "tm1b04")
        w2maT = load_const(w2maT_d, [128, 128], "w2maT")
        w2mbT = load_const(w2mbT_d, [128, 128], "w2mbT")
        tm206 = load_const(tm206_d, [128, 1], "tm206")
        tm204 = load_const(tm204_d, [128, 1], "tm204")
        w3T = load_const(w3T_d, [128, 1], "w3T")
        b3 = load_const(b3_d, [1, 1], "b3")
        augb1 = load_const(augb1_d, [128, 1], "augb1")
        augb2 = load_const(augb2_d, [128, 1], "augb2")

        At = dpool.tile([n, C], f32, tag="At")
        xaug = xpool.tile([C + 2, n], f32, tag="xaug")
        x2aug = xpool.tile([C + 2, n], f32, tag="x2aug")
        Bt = xpool.tile([C, n], f32, tag="Bt")       # B' channel-major
        iota9 = xpool.tile([128, n], u32, tag="iota9")
        packed = xpool.tile([128, n], u32, tag="packed")
        H = hpool.tile([C, n], f32, tag="H")
        osb = hpool.tile([1, n], f32, tag="osb")

        def r32(ap):
            # walrus BIR verifier rejects f32r matmul inputs that are not
            # produced by an f32r-rounding op (e.g. raw DMA loads), so run
            # all matmuls in plain fp32.
            return ap

        # segment-local iota (col mod 512), same on all partitions
        for s in range(8):
            nc.gpsimd.iota(iota9[:, 512 * s:512 * (s + 1)], pattern=[[1, 512]],
                           base=0, channel_multiplier=0)

        # ---------------- stage 0: tables ----------------
        with tc.tile_pool(name="s0sb", bufs=2) as s0sb, \
             tc.tile_pool(name="s0ps", bufs=3, space="PSUM") as s0ps:
            nc.sync.dma_start(xaug[:C, :], x_d[:])
            nc.scalar.activation(out=x2aug[:C, :], in_=xaug[:C, :],
                                 func=AF.Copy, scale=2.0)
            nc.scalar.activation(out=x2aug[C:C + 2, :], in_=xaug[0:2, :],
                                 func=AF.Identity, bias=augb1[C:C + 2],
                                 scale=0.0)
            nc.scalar.activation(out=xaug[C:C + 2, :], in_=xaug[0:2, :],
                                 func=AF.Identity, bias=augb2[C:C + 2],
                                 scale=0.0)
            for ch in range(nchk):
                cs = slice(512 * ch, 512 * (ch + 1))
                xsq = s0sb.tile([C, 512], f32, tag="xsq")
                nc.scalar.activation(out=xsq[:], in_=xaug[:C, cs], func=AF.Square)
                psxx = s0ps.tile([1, 512], f32, tag="s0p", space="PSUM")
                nc.tensor.matmul(out=psxx[:], lhsT=ones64[:], rhs=xsq[:],
                                 start=True, stop=True)
                nc.scalar.copy(out=xaug[C:C + 1, cs], in_=psxx[:])
            for ch in range(nchk):
                cs = slice(512 * ch, 512 * (ch + 1))
                # B' chunk: channel-major, bias t1 folded
                psb = s0ps.tile([C, 512], f32, tag="s0p", space="PSUM")
                nc.tensor.matmul(out=psb[:], lhsT=r32(wcnT[:]),
                                 rhs=r32(xaug[:C, cs]), start=True, stop=True)
                nc.scalar.activation(out=Bt[:, cs], in_=psb[:], func=AF.Identity,
                                     bias=t1[:], scale=1.0)
                # A' chunk then transpose to DRAM table
                psa = s0ps.tile([C, 512], f32, tag="s0p", space="PSUM")
                nc.tensor.matmul(out=psa[:], lhsT=r32(wnT[:]),
                                 rhs=r32(xaug[:C, cs]), start=True, stop=True)
                ap = s0sb.tile([C, 512], f32, tag="ap")
                nc.scalar.copy(out=ap[:], in_=psa[:])
                for j in range(4):
                    blk = 4 * ch + j
                    js = slice(128 * j, 128 * (j + 1))
                    pta = s0ps.tile([128, C], f32, tag="s0p", space="PSUM")
                    nc.tensor.transpose(out=pta[:], in_=ap[:, js],
                                        identity=ident[:C, :C])
                    ast = s0sb.tile([128, C], f32, tag="ast")
                    nc.scalar.copy(out=ast[:], in_=pta[:])
                    nc.sync.dma_start(At[128 * blk:128 * (blk + 1), :], ast[:])

        # ---------------- stage 1 ----------------
        with tc.tile_pool(name="rps", bufs=2, space="PSUM") as rps, \
             tc.tile_pool(name="eps", bufs=2, space="PSUM") as eps, \
             tc.tile_pool(name="cps", bufs=2, space="PSUM") as cps, \
             tc.tile_pool(name="mps", bufs=1, space="PSUM") as mps, \
             tc.tile_pool(name="gpool", bufs=2) as gpool, \
             tc.tile_pool(name="vpool", bufs=2) as vpool, \
             tc.tile_pool(name="zpool", bufs=2) as zpool, \
             tc.tile_pool(name="wpool", bufs=2) as wpool, \
             tc.tile_pool(name="mpool", bufs=2) as mpool:

            idx_t = {}

            def emit_A(b):
                """pairwise (PE, f32r) + pack (gpsimd, from PSUM)."""
                bs = slice(128 * b, 128 * (b + 1))
                for q in range(8):
                    qs = slice(512 * q, 512 * (q + 1))
                    ps = rps.tile([128, 512], f32, tag="r", space="PSUM")
                    nc.tensor.matmul(out=ps[:], lhsT=r32(x2aug[:, bs]),
                                     rhs=r32(xaug[:, qs]), start=True, stop=True)
                    rsb = vpool.tile([128, 512], f32, tag="rsb")
                    nc.scalar.copy(out=rsb[:], in_=ps[:])
                    nc.gpsimd.scalar_tensor_tensor(
                        out=packed[:, qs], in0=rsb[:].bitcast(u32),
                        scalar=0xFFFFFE00, in1=iota9[:, qs],
                        op0=OP.bitwise_and, op1=OP.bitwise_or)

            def emit_B(b):
                """segmented top-k + candidate merge + decode (DVE)."""
                cand = vpool.tile([128, 64], u32, tag="cand")
                for s in range(8):
                    nc.vector.max(out=cand[:, 8 * s:8 * (s + 1)],
                                  in_=packed[:, 512 * s:512 * (s + 1)])
                win = vpool.tile([128, 24], u32, tag="win")
                pos = vpool.tile([128, 24], u32, tag="pos")
                for r3 in range(3):
                    sl = slice(8 * r3, 8 * (r3 + 1))
                    nc.vector.max(out=win[:, sl], in_=cand[:])
                    nc.vector.max_index(out=pos[:, sl], in_max=win[:, sl],
                                        in_values=cand[:])
                    if r3 < 2:
                        nc.vector.match_replace(out=cand[:],
                                                in_to_replace=win[:, sl],
                                                in_values=cand[:],
                                                imm_value=NEG_FILL)
                seg = vpool.tile([128, 24], u32, tag="seg")
                nc.vector.tensor_scalar(out=seg[:], in0=pos[:], scalar1=3,
                                        scalar2=9, op0=OP.logical_shift_right,
                                        op1=OP.logical_shift_left)
                gl = vpool.tile([128, 24], u32, tag="gl")
                nc.vector.scalar_tensor_tensor(out=gl[:], in0=win[:], scalar=511,
                                               in1=seg[:], op0=OP.bitwise_and,
                                               op1=OP.bitwise_or)
                idx_t[b] = gl

            def emit_C(b):
                """one multi-index gather (gpsimd)."""
                gl = idx_t.pop(b)
                G = gpool.tile([128, K * C], f32, tag="G")
                nc.gpsimd.indirect_dma_start(
                    out=G[:].rearrange("p (k c) -> p k c", k=K), out_offset=None,
                    in_=At[:],
                    in_offset=bass.IndirectOffsetOnAxis(ap=gl[:, :K], axis=0))
                return G

            def emit_D(b, G):
                """edge transposes + B-add + conv2 with folded lrelu."""
                bs = slice(128 * b, 128 * (b + 1))
                bt4 = wpool.tile([C, 512], f32, tag="bt4")
                nc.scalar.activation(
                    out=bt4[:].rearrange("c (a p) -> c a p", a=4),
                    in_=Bt[:, bs].rearrange("c (a p) -> c a p", a=1)
                        .to_broadcast([C, 4, 128]),
                    func=AF.Copy, scale=1.0)
                ew = mpool.tile([128, 10 * 128], f32, tag="ew")
                for g in range(3):          # k pairs: 4, 4, 2
                    npair = 4 if g < 2 else 2
                    w = 128 * npair
                    ep = eps.tile([128, 512], f32, tag="e", space="PSUM")
                    nc.tensor.matmul(out=ep[:, :w], lhsT=r32(i2[:]),
                                     rhs=r32(bt4[:, :w]), start=True, stop=False)
                    for t in range(npair):
                        nc.tensor.matmul(
                            out=r32(ep[:, 128 * t:128 * (t + 1)]),
                            lhsT=r32(G[:, 128 * (4 * g + t):128 * (4 * g + t + 1)]),
                            rhs=r32(ident[:]), is_transpose=True,
                            start=False, stop=(t == npair - 1))
                    zs = zpool.tile([128, 512], f32, tag="zs")
                    za = zpool.tile([128, 512], f32, tag="za")
                    nc.scalar.activation(out=zs[:, :w], in_=ep[:, :w],
                                         func=AF.Copy, scale=0.6)
                    nc.scalar.activation(out=za[:, :w], in_=ep[:, :w],
                                         func=AF.Abs, scale=0.4)
                    cp = cps.tile([128, 512], f32, tag="c", space="PSUM")
                    nc.tensor.matmul(out=cp[:C, :w], lhsT=r32(w2T[:C, :]),
                                     rhs=r32(zs[:C, :w]), start=True, stop=False)
                    nc.tensor.matmul(out=cp[:C, :w], lhsT=r32(w2T[:C, :]),
                                     rhs=r32(za[:C, :w]), start=False, stop=True)
                    nc.tensor.matmul(out=cp[C:, :w], lhsT=r32(w2T[C:, :]),
                                     rhs=r32(zs[C:, :w]), start=True, stop=False)
                    nc.tensor.matmul(out=cp[C:, :w], lhsT=r32(w2T[C:, :]),
                                     rhs=r32(za[C:, :w]), start=False, stop=True)
                    gs = slice(512 * g, 512 * g + w)
                    nc.scalar.copy(out=ew[:, gs], in_=cp[:, :w])
                return ew

            def emit_E(b, ew):
                """max over k (both parity halves), +t2, lrelu -> H."""
                bs = slice(128 * b, 128 * (b + 1))
                m4 = mpool.tile([C, 128], f32, tag="m4")
                nc.vector.tensor_reduce(
                    out=m4[:],
                    in_=ew[:C, :].rearrange("c (k p) -> c p k", k=10),
                    axis=AX.X, op=OP.max)
                mo = mpool.tile([C, 128], f32, tag="mo")
                nc.vector.tensor_reduce(
                    out=mo[:],
                    in_=ew[C:, :].rearrange("c (k p) -> c p k", k=10),
                    axis=AX.X, op=OP.max)
                z = mpool.tile([C, 128], f32, tag="z")
                nc.gpsimd.tensor_tensor(out=z[:], in0=m4[:], in1=mo[:], op=OP.max)
                nc.gpsimd.tensor_scalar(out=z[:], in0=z[:], scalar1=t2[:],
                                        scalar2=None, op0=OP.add)
                nc.gpsimd.scalar_tensor_tensor(out=H[:, bs], in0=z[:], scalar=NEG,
                                               in1=z[:], op0=OP.mult, op1=OP.max)

            def emit_MLP(ch):
                cs = slice(512 * ch, 512 * (ch + 1))
                zdr = []
                for wT, t06, t04 in ((w1aT, tm1a06, tm1a04),
                                     (w1bT, tm1b06, tm1b04)):
                    p1 = mps.tile([128, 512], f32, tag="m", space="PSUM")
                    nc.tensor.matmul(out=p1[:], lhsT=r32(wT[:]),
                                     rhs=r32(H[:, cs]), start=True, stop=True)
                    zs1 = zpool.tile([128, 512], f32, tag="zs1")
                    za1 = zpool.tile([128, 512], f32, tag="za1")
                    nc.scalar.activation(out=zs1[:], in_=p1[:], func=AF.Identity,
                                         bias=t06[:], scale=0.6)
                    nc.scalar.activation(out=za1[:], in_=p1[:], func=AF.Abs,
                                         bias=t04[:], scale=0.4)
                    zdr.append((zs1, za1))
                p2 = mps.tile([128, 512], f32, tag="m", space="PSUM")
                mms = [(w2maT, zdr[0][0]), (w2maT, zdr[0][1]),
                       (w2mbT, zdr[1][0]), (w2mbT, zdr[1][1])]
                for j, (wT, rh) in enumerate(mms):
                    nc.tensor.matmul(out=p2[:], lhsT=r32(wT[:]), rhs=r32(rh[:]),
                                     start=(j == 0), stop=(j == 3))
                zs2 = zpool.tile([128, 512], f32, tag="zs2")
                za2 = zpool.tile([128, 512], f32, tag="za2")
                nc.scalar.activation(out=zs2[:], in_=p2[:], func=AF.Identity,
                                     bias=tm206[:], scale=0.6)
                nc.scalar.activation(out=za2[:], in_=p2[:], func=AF.Abs,
                                     bias=tm204[:], scale=0.4)
                p3 = mps.tile([1, 512], f32, tag="m3", space="PSUM")
                nc.tensor.matmul(out=p3[:], lhsT=r32(w3T[:]), rhs=r32(zs2[:]),
                                 start=True, stop=False)
                nc.tensor.matmul(out=p3[:], lhsT=r32(w3T[:]), rhs=r32(za2[:]),
                                 start=False, stop=True)
                nc.scalar.activation(out=osb[:, cs], in_=p3[:], func=AF.Identity,
                                     bias=b3[:], scale=1.0)

            emit_A(0)
            emit_B(0)
            G = {}
            for b in range(nblk):
                if b + 1 < nblk:
                    emit_A(b + 1)
                Gb = emit_C(b)
                if b + 1 < nblk:
                    emit_B(b + 1)
                ew = emit_D(b, Gb)
                emit_E(b, ew)
                if b % 4 == 3:
                    emit_MLP(b // 4)
            nc.sync.dma_start(out_d[:], osb[:])

    nc.finalize()
    return nc


def host_weights(w_k1, g_k1, b_k1, m_k1, v_k1, w_k2, g_k2, b_k2, m_k2, v_k2,
                 w1, g1, b1, m1, v1, w2, g2, b2, m2, v2, w3, b3):
    f = np.float32
    s1 = (g_k1 / np.sqrt(v_k1 + f(EPS))).astype(f)
    t1 = (b_k1 - m_k1 * s1).astype(f)
    wn = w_k1[:, :C]
    wc = w_k1[:, C:]
    wnT = np.ascontiguousarray((wn * s1[:, None]).T.astype(f))
    wcnT = np.ascontiguousarray(((wc - wn) * s1[:, None]).T.astype(f))
    s2 = (g_k2 / np.sqrt(v_k2 + f(EPS))).astype(f)
    t2 = (b_k2 - m_k2 * s2).astype(f)
    w2T = np.ascontiguousarray((w_k2 * s2[:, None]).T.astype(f))
    augb1 = np.zeros((128, 1), f)
    augb1[C, 0] = -1.0
    augb1[C + 1, 0] = 1.0
    augb2 = np.zeros((128, 1), f)
    augb2[C + 1, 0] = OFF
    sm1 = (g1 / np.sqrt(v1 + f(EPS))).astype(f)
    tm1 = (b1 - m1 * sm1).astype(f)
    w1s = (w1 * sm1[:, None]).astype(f)
    w1aT = np.ascontiguousarray(w1s[:128].T)
    w1bT = np.ascontiguousarray(w1s[128:].T)
    sm2 = (g2 / np.sqrt(v2 + f(EPS))).astype(f)
    tm2 = (b2 - m2 * sm2).astype(f)
    w2s = (w2 * sm2[:, None]).astype(f)
    w2maT = np.ascontiguousarray(w2s[:, :128].T)
    w2mbT = np.ascontiguousarray(w2s[:, 128:].T)
    w3T = np.ascontiguousarray(w3.T.astype(f))
    return {
        "wnT": wnT, "wcnT": wcnT, "t1": t1.reshape(C, 1),
        "w2T": w2T, "t2": t2.reshape(C, 1),
        "w1aT": w1aT, "w1bT": w1bT,
        "tm1a06": (0.6 * tm1[:128]).reshape(128, 1).astype(f),
        "tm1a04": (0.4 * tm1[:128]).reshape(128, 1).astype(f),
        "tm1b06": (0.6 * tm1[128:]).reshape(128, 1).astype(f),
        "tm1b04": (0.4 * tm1[128:]).reshape(128, 1).astype(f),
        "w2maT": w2maT, "w2mbT": w2mbT,
        "tm206": (0.6 * tm2).reshape(128, 1).astype(f),
        "tm204": (0.4 * tm2).reshape(128, 1).astype(f),
        "w3T": w3T, "b3": b3.reshape(1, 1).astype(f),
        "augb1": augb1, "augb2": augb2,
    }


def kernel(**inputs):
    from concourse.bass_utils import run_bass_kernel_spmd

    x = np.asarray(inputs["x"], dtype=np.float32)
    B = x.shape[0]
    n = x.shape[2]
    w = host_weights(**{k: np.asarray(v, dtype=np.float32)
                        for k, v in inputs.items() if k != "x"})
    if n not in _cache:
        _cache[n] = build_nc(n)
    nc = _cache[n]
    in_maps = [{"x": np.ascontiguousarray(x[c]), **w} for c in range(B)]
    res = run_bass_kernel_spmd(nc, in_maps, list(range(NCORES)))
    out = np.stack([res.results[c]["out"][0] for c in range(B)], axis=0)
    return out.astype(np.float32)
